# revision 8
# baseline (speedup 1.0000x reference)
"""Brute-force KNN retrieval (B=512 queries, N=500000 candidates, D=128, top-K)
on 8 Trainium2 NeuronCores.

Strategy (sharding_hint): candidates sharded along N across the 8 cores,
queries replicated. Per core:
  - PE computes bf16 scores (fp32 PSUM) for its 62500-candidate shard.
  - ACT casts each 2048-wide PSUM chunk to fp16(score+128) and writes it
    into the HIGH int16 lanes of a persistent fp32 "packed" scan tile whose
    LOW int16 lanes hold a one-time iota (0..2047). For positive floats the
    fp32 bit pattern is monotone, so each packed fp32 orders by
    (fp16 score, then index).
  - DVE max8 reduces each packed chunk to its top-8 (values AND indices in
    one pass - no max_index / second scan needed).
Keep-top-8-per-2048-chunk is a safe reduction for top-100-of-500000
(P[>8 of a row's top-100 in one chunk] ~ 2e-8, plus tiny fp16-tie effects).
The host decodes survivors, rescores the top ~256 per row exactly in fp32,
and emits the exact global top-K (ties -> lower index, like lax.top_k).
"""

import sys

for _p in ("/opt/trn_rl_repo",):
    if _p not in sys.path:
        sys.path.insert(0, _p)

import numpy as np

B, N, D = 512, 500000, 128
N_CORES = 8
SHARD = N // N_CORES          # 62500 candidates per core
PCHUNK = 2048                 # PSUM tile width (4 banks) = max8 chunk
NCHUNK = -(-SHARD // PCHUNK)  # 31
PADN = PCHUNK * NCHUNK        # 63488 (padded shard width)
NSUB = PCHUNK // 512          # 4 matmuls per PSUM tile
MTILES = B // 128             # 4 query tiles
SURV = NCHUNK * 8             # 248 survivors per (row, core)
SCAN_BUFS = 10                 # persistent packed scan tiles (iota-carrying)
RESCORE = 256                 # host rescores this many per row exactly
BIAS = 128.0                  # score bias -> positive range for bit-ordering

_NC_CACHE = {}


def _build_nc():
    import concourse.bacc as bacc
    import concourse.tile as tile
    import concourse.mybir as mybir

    f32 = mybir.dt.float32
    f16 = mybir.dt.float16
    u16 = mybir.dt.uint16
    bf16 = mybir.dt.bfloat16

    nc = bacc.Bacc(
        "TRN2", target_bir_lowering=False, debug=False, num_devices=N_CORES
    )
    qT = nc.dram_tensor("qT", [D, B], bf16, kind="ExternalInput")
    cT = nc.dram_tensor("cT", [D, PADN], bf16, kind="ExternalInput")
    si = nc.dram_tensor("si", [128, PCHUNK], f32, kind="ExternalInput")
    packed = nc.dram_tensor("packed", [B, SURV], f32, kind="ExternalOutput")

    with tile.TileContext(nc) as tc:
        with (
            tc.tile_pool(name="q", bufs=1) as qp,
            tc.tile_pool(name="c", bufs=4) as cp,
            tc.tile_pool(name="ps", bufs=2, space="PSUM") as pp,
            tc.tile_pool(name="scan", bufs=1) as sp,
            tc.tile_pool(name="acc", bufs=1) as op,
        ):
            qt = qp.tile([128, B], bf16)
            nc.sync.dma_start(qt[:], qT.ap())

            pacc = [
                op.tile([128, SURV], f32, name=f"pacc{m}", tag=f"p{m}")
                for m in range(MTILES)
            ]
            scan = [
                sp.tile([128, PCHUNK], f32, name=f"scan{j}", tag=f"s{j}")
                for j in range(SCAN_BUFS)
            ]
            # one-time iota image (idx in LOW int16 lane of each packed fp32),
            # DMA'd so startup doesn't serialize on GPSIMD iota ops
            for j in range(SCAN_BUFS):
                nc.sync.dma_start(scan[j][:], si.ap())

            for c in range(NCHUNK):
                ct = cp.tile([128, PCHUNK], bf16, name=f"ct{c}", tag="ct")
                nc.sync.dma_start(ct[:], cT.ap()[:, c * PCHUNK:(c + 1) * PCHUNK])
                for m in range(MTILES):
                    ps = pp.tile([128, PCHUNK], f32, name=f"ps{c}_{m}", tag="ps")
                    for s in range(NSUB):
                        nc.tensor.matmul(
                            ps[:, s * 512:(s + 1) * 512],
                            qt[:, m * 128:(m + 1) * 128],
                            ct[:, s * 512:(s + 1) * 512],
                            start=True,
                            stop=True,
                        )
                    sj = scan[(c * MTILES + m) % SCAN_BUFS]
                    hi = sj[:].bitcast(f16).rearrange(
                        "p (n two) -> p n two", two=2
                    )[:, :, 1]
                    nc.scalar.activation(
                        hi, ps[:], mybir.ActivationFunctionType.Copy,
                        bias=BIAS, scale=1.0,
                    )
                    nc.vector.max(pacc[m][:, c * 8:(c + 1) * 8], sj[:])

            for m in range(MTILES):
                nc.sync.dma_start(packed.ap()[m * 128:(m + 1) * 128, :], pacc[m][:])

    nc.compile()
    return nc


def _get_nc():
    if "nc" not in _NC_CACHE:
        _NC_CACHE["nc"] = _build_nc()
    return _NC_CACHE["nc"]


def _scan_init():
    # packed-fp32 scan-tile image: iota in the low int16 lane, zero high lane
    lo = np.arange(PCHUNK, dtype=np.uint32)          # hi 16 bits zero
    return np.ascontiguousarray(
        np.broadcast_to(lo, (128, PCHUNK))
    ).view(np.float32)


def _make_in_maps(queries, candidates):
    import ml_dtypes

    bf = ml_dtypes.bfloat16
    q = np.asarray(queries, dtype=np.float32)
    cand = np.asarray(candidates, dtype=np.float32)
    qTh = np.ascontiguousarray(q.T.astype(bf))  # [D, B] bf16
    in_maps = []
    for i in range(N_CORES):
        cTi = np.zeros((D, PADN), dtype=bf)
        cTi[:, :SHARD] = cand[i * SHARD:(i + 1) * SHARD].T.astype(bf)
        in_maps.append({"qT": qTh, "cT": cTi, "si": _scan_init()})
    return in_maps


def _run_device(in_maps, trace=False):
    from concourse import bass_utils

    nc = _get_nc()
    return bass_utils.run_bass_kernel_spmd(
        nc, in_maps, core_ids=list(range(N_CORES)), trace=trace
    )


def _merge(results, queries, candidates, identifiers, num_candidates):
    K = int(num_candidates)
    q = np.asarray(queries, dtype=np.float32)
    cand = np.asarray(candidates, dtype=np.float32)
    chunk_base = np.repeat(np.arange(NCHUNK, dtype=np.int64) * PCHUNK, 8)  # [SURV]
    all_u = []
    all_g = []
    for i in range(N_CORES):
        u = np.asarray(results[i]["packed"]).view(np.uint32)       # [B, SURV]
        local = chunk_base[None, :] + (u & 0xFFFF)                 # [B, SURV]
        valid = local < SHARD
        u = np.where(valid, u, 0)  # pads rank last
        g = i * SHARD + np.minimum(local, SHARD - 1)
        all_u.append(u)
        all_g.append(g)
    ucat = np.concatenate(all_u, axis=1)   # [B, 8*SURV] packed (monotone rank)
    gcat = np.concatenate(all_g, axis=1)
    # candidate set for exact rescoring: top RESCORE per row by packed rank
    nres = min(RESCORE, ucat.shape[1])
    part = np.argpartition(ucat, ucat.shape[1] - nres, axis=1)[:, -nres:]
    rows = np.arange(B)[:, None]
    gsel = gcat[rows, part]                                        # [B, nres]
    # exact fp32 rescore: s[b, j] = q[b] . cand[gsel[b, j]]
    csel = cand[gsel]                                              # [B, nres, D]
    vsel = np.einsum("bjd,bd->bj", csel, q, dtype=np.float32)
    # exact top-K, ties -> lower global index (matches lax.top_k)
    order = np.lexsort((gsel, -vsel), axis=-1)[:, :K]
    out_vals = np.take_along_axis(vsel, order, axis=1).astype(np.float32)
    out_gidx = np.take_along_axis(gsel, order, axis=1)
    ids = np.asarray(identifiers)
    out_ids = np.take(ids, out_gidx, axis=0)
    return out_vals, out_ids


def kernel(queries, candidates, identifiers, num_candidates):
    in_maps = _make_in_maps(queries, candidates)
    res = _run_device(in_maps, trace=False)
    return _merge(res.results, queries, candidates, identifiers, num_candidates)


# revision 9
# speedup vs baseline: 1.0769x; 1.0769x over previous
"""Brute-force KNN retrieval (B=512 queries, N=500000 candidates, D=128, top-K)
on 8 Trainium2 NeuronCores.

Strategy (sharding_hint): candidates sharded along N across the 8 cores,
queries replicated. Per core:
  - PE computes bf16 scores (fp32 PSUM) for its 62500-candidate shard.
  - ACT casts each 2048-wide PSUM chunk to fp16(score+128) and writes it
    into the HIGH int16 lanes of a persistent fp32 "packed" scan tile whose
    LOW int16 lanes hold a one-time iota (0..2047). For positive floats the
    fp32 bit pattern is monotone, so each packed fp32 orders by
    (fp16 score, then index).
  - DVE max8 reduces each packed chunk to its top-8 (values AND indices in
    one pass - no max_index / second scan needed).
Keep-top-8-per-2048-chunk is a safe reduction for top-100-of-500000
(P[>8 of a row's top-100 in one chunk] ~ 2e-8, plus tiny fp16-tie effects).
The host decodes survivors, rescores the top ~256 per row exactly in fp32,
and emits the exact global top-K (ties -> lower index, like lax.top_k).
"""

import sys

for _p in ("/opt/trn_rl_repo",):
    if _p not in sys.path:
        sys.path.insert(0, _p)

import numpy as np

B, N, D = 512, 500000, 128
N_CORES = 8
SHARD = N // N_CORES          # 62500 candidates per core
PCHUNK = 2048                 # PSUM tile width (4 banks) = max8 chunk
NCHUNK = -(-SHARD // PCHUNK)  # 31
PADN = PCHUNK * NCHUNK        # 63488 (padded shard width)
NSUB = PCHUNK // 512          # 4 matmuls per PSUM tile
MTILES = B // 128             # 4 query tiles
SURV = NCHUNK * 8             # 248 survivors per (row, core)
SCAN_BUFS = 8                 # persistent packed scan tiles (iota-carrying)
RESCORE = 256                 # host rescores this many per row exactly
BIAS = 128.0                  # score bias -> positive range for bit-ordering

_NC_CACHE = {}


def _build_nc():
    import concourse.bacc as bacc
    import concourse.tile as tile
    import concourse.mybir as mybir

    f32 = mybir.dt.float32
    f16 = mybir.dt.float16
    u16 = mybir.dt.uint16
    bf16 = mybir.dt.bfloat16

    nc = bacc.Bacc(
        "TRN2", target_bir_lowering=False, debug=False, num_devices=N_CORES
    )
    qT = nc.dram_tensor("qT", [D, B], bf16, kind="ExternalInput")
    cT = nc.dram_tensor("cT", [D, PADN], bf16, kind="ExternalInput")
    packed = nc.dram_tensor("packed", [B, SURV], f32, kind="ExternalOutput")

    with tile.TileContext(nc) as tc:
        with (
            tc.tile_pool(name="q", bufs=1) as qp,
            tc.tile_pool(name="c", bufs=4) as cp,
            tc.tile_pool(name="ps", bufs=2, space="PSUM") as pp,
            tc.tile_pool(name="scan", bufs=1) as sp,
            tc.tile_pool(name="acc", bufs=1) as op,
        ):
            qt = qp.tile([128, B], bf16)
            nc.sync.dma_start(qt[:], qT.ap())

            pacc = [
                op.tile([128, SURV], f32, name=f"pacc{m}", tag=f"p{m}")
                for m in range(MTILES)
            ]
            scan = [
                sp.tile([128, PCHUNK], f32, name=f"scan{j}", tag=f"s{j}")
                for j in range(SCAN_BUFS)
            ]
            # one-time iota into the LOW int16 lane of each packed fp32
            for j in range(SCAN_BUFS):
                lo = scan[j][:].bitcast(u16).rearrange(
                    "p (n two) -> p n two", two=2
                )[:, :, 0]
                nc.gpsimd.iota(lo, pattern=[[1, PCHUNK]], base=0,
                               channel_multiplier=0)

            for c in range(NCHUNK):
                ct = cp.tile([128, PCHUNK], bf16, name=f"ct{c}", tag="ct")
                nc.sync.dma_start(ct[:], cT.ap()[:, c * PCHUNK:(c + 1) * PCHUNK])
                for m in range(MTILES):
                    ps = pp.tile([128, PCHUNK], f32, name=f"ps{c}_{m}", tag="ps")
                    for s in range(NSUB):
                        nc.tensor.matmul(
                            ps[:, s * 512:(s + 1) * 512],
                            qt[:, m * 128:(m + 1) * 128],
                            ct[:, s * 512:(s + 1) * 512],
                            start=True,
                            stop=True,
                        )
                    sj = scan[(c * MTILES + m) % SCAN_BUFS]
                    hi = sj[:].bitcast(f16).rearrange(
                        "p (n two) -> p n two", two=2
                    )[:, :, 1]
                    nc.scalar.activation(
                        hi, ps[:], mybir.ActivationFunctionType.Copy,
                        bias=BIAS, scale=1.0,
                    )
                    nc.vector.max(pacc[m][:, c * 8:(c + 1) * 8], sj[:])

            for m in range(MTILES):
                nc.sync.dma_start(packed.ap()[m * 128:(m + 1) * 128, :], pacc[m][:])

    nc.compile()
    return nc


def _get_nc():
    if "nc" not in _NC_CACHE:
        _NC_CACHE["nc"] = _build_nc()
    return _NC_CACHE["nc"]


def _make_in_maps(queries, candidates):
    import ml_dtypes

    bf = ml_dtypes.bfloat16
    q = np.asarray(queries, dtype=np.float32)
    cand = np.asarray(candidates, dtype=np.float32)
    qTh = np.ascontiguousarray(q.T.astype(bf))  # [D, B] bf16
    in_maps = []
    for i in range(N_CORES):
        cTi = np.zeros((D, PADN), dtype=bf)
        cTi[:, :SHARD] = cand[i * SHARD:(i + 1) * SHARD].T.astype(bf)
        in_maps.append({"qT": qTh, "cT": cTi})
    return in_maps


def _run_device(in_maps, trace=False):
    from concourse import bass_utils

    nc = _get_nc()
    return bass_utils.run_bass_kernel_spmd(
        nc, in_maps, core_ids=list(range(N_CORES)), trace=trace
    )


def _merge(results, queries, candidates, identifiers, num_candidates):
    K = int(num_candidates)
    q = np.asarray(queries, dtype=np.float32)
    cand = np.asarray(candidates, dtype=np.float32)
    chunk_base = np.repeat(np.arange(NCHUNK, dtype=np.int64) * PCHUNK, 8)  # [SURV]
    all_u = []
    all_g = []
    for i in range(N_CORES):
        u = np.asarray(results[i]["packed"]).view(np.uint32)       # [B, SURV]
        local = chunk_base[None, :] + (u & 0xFFFF)                 # [B, SURV]
        valid = local < SHARD
        u = np.where(valid, u, 0)  # pads rank last
        g = i * SHARD + np.minimum(local, SHARD - 1)
        all_u.append(u)
        all_g.append(g)
    ucat = np.concatenate(all_u, axis=1)   # [B, 8*SURV] packed (monotone rank)
    gcat = np.concatenate(all_g, axis=1)
    # candidate set for exact rescoring: top RESCORE per row by packed rank
    nres = min(RESCORE, ucat.shape[1])
    part = np.argpartition(ucat, ucat.shape[1] - nres, axis=1)[:, -nres:]
    rows = np.arange(B)[:, None]
    gsel = gcat[rows, part]                                        # [B, nres]
    # exact fp32 rescore: s[b, j] = q[b] . cand[gsel[b, j]]
    csel = cand[gsel]                                              # [B, nres, D]
    vsel = np.einsum("bjd,bd->bj", csel, q, dtype=np.float32)
    # exact top-K, ties -> lower global index (matches lax.top_k)
    order = np.lexsort((gsel, -vsel), axis=-1)[:, :K]
    out_vals = np.take_along_axis(vsel, order, axis=1).astype(np.float32)
    out_gidx = np.take_along_axis(gsel, order, axis=1)
    ids = np.asarray(identifiers)
    out_ids = np.take(ids, out_gidx, axis=0)
    return out_vals, out_ids


def kernel(queries, candidates, identifiers, num_candidates):
    in_maps = _make_in_maps(queries, candidates)
    res = _run_device(in_maps, trace=False)
    return _merge(res.results, queries, candidates, identifiers, num_candidates)


# revision 10
# speedup vs baseline: 1.0794x; 1.0024x over previous
"""Brute-force KNN retrieval (B=512 queries, N=500000 candidates, D=128, top-K)
on 8 Trainium2 NeuronCores.

Strategy (sharding_hint): candidates sharded along N across the 8 cores,
queries replicated. Per core:
  - PE computes bf16 scores (fp32 PSUM) for its 62500-candidate shard.
  - ACT casts each 2048-wide PSUM chunk to fp16(score+128) and writes it
    into the HIGH int16 lanes of a persistent fp32 "packed" scan tile whose
    LOW int16 lanes hold a one-time iota (0..2047). For positive floats the
    fp32 bit pattern is monotone, so each packed fp32 orders by
    (fp16 score, then index).
  - DVE max8 reduces each packed chunk to its top-8 (values AND indices in
    one pass - no max_index / second scan needed).
Keep-top-8-per-2048-chunk is a safe reduction for top-100-of-500000
(P[>8 of a row's top-100 in one chunk] ~ 2e-8, plus tiny fp16-tie effects).
The host decodes survivors, rescores the top ~256 per row exactly in fp32,
and emits the exact global top-K (ties -> lower index, like lax.top_k).
"""

import sys

for _p in ("/opt/trn_rl_repo",):
    if _p not in sys.path:
        sys.path.insert(0, _p)

import numpy as np

B, N, D = 512, 500000, 128
N_CORES = 8
SHARD = N // N_CORES          # 62500 candidates per core
PCHUNK = 2048                 # PSUM tile width (4 banks) = max8 chunk
NCHUNK = -(-SHARD // PCHUNK)  # 31
PADN = PCHUNK * NCHUNK        # 63488 (padded shard width)
NSUB = PCHUNK // 512          # 4 matmuls per PSUM tile
MTILES = B // 128             # 4 query tiles
SURV = NCHUNK * 8             # 248 survivors per (row, core)
SCAN_BUFS = 4                 # persistent packed scan tiles (iota-carrying)
RESCORE = 256                 # host rescores this many per row exactly
BIAS = 128.0                  # score bias -> positive range for bit-ordering

_NC_CACHE = {}


def _build_nc():
    import concourse.bacc as bacc
    import concourse.tile as tile
    import concourse.mybir as mybir

    f32 = mybir.dt.float32
    f16 = mybir.dt.float16
    u16 = mybir.dt.uint16
    bf16 = mybir.dt.bfloat16

    nc = bacc.Bacc(
        "TRN2", target_bir_lowering=False, debug=False, num_devices=N_CORES
    )
    qT = nc.dram_tensor("qT", [D, B], bf16, kind="ExternalInput")
    cT = nc.dram_tensor("cT", [D, PADN], bf16, kind="ExternalInput")
    packed = nc.dram_tensor("packed", [B, SURV], f32, kind="ExternalOutput")

    with tile.TileContext(nc) as tc:
        with (
            tc.tile_pool(name="q", bufs=1) as qp,
            tc.tile_pool(name="c", bufs=4) as cp,
            tc.tile_pool(name="ps", bufs=2, space="PSUM") as pp,
            tc.tile_pool(name="scan", bufs=1) as sp,
            tc.tile_pool(name="acc", bufs=1) as op,
        ):
            qt = qp.tile([128, B], bf16)
            nc.sync.dma_start(qt[:], qT.ap())

            pacc = [
                op.tile([128, SURV], f32, name=f"pacc{m}", tag=f"p{m}")
                for m in range(MTILES)
            ]
            scan = [
                sp.tile([128, PCHUNK], f32, name=f"scan{j}", tag=f"s{j}")
                for j in range(SCAN_BUFS)
            ]
            # one-time iota into the LOW int16 lane of each packed fp32
            for j in range(SCAN_BUFS):
                lo = scan[j][:].bitcast(u16).rearrange(
                    "p (n two) -> p n two", two=2
                )[:, :, 0]
                nc.gpsimd.iota(lo, pattern=[[1, PCHUNK]], base=0,
                               channel_multiplier=0)

            for c in range(NCHUNK):
                ct = cp.tile([128, PCHUNK], bf16, name=f"ct{c}", tag="ct")
                nc.sync.dma_start(ct[:], cT.ap()[:, c * PCHUNK:(c + 1) * PCHUNK])
                for m in range(MTILES):
                    ps = pp.tile([128, PCHUNK], f32, name=f"ps{c}_{m}", tag="ps")
                    for s in range(NSUB):
                        nc.tensor.matmul(
                            ps[:, s * 512:(s + 1) * 512],
                            qt[:, m * 128:(m + 1) * 128],
                            ct[:, s * 512:(s + 1) * 512],
                            start=True,
                            stop=True,
                        )
                    sj = scan[(c * MTILES + m) % SCAN_BUFS]
                    hi = sj[:].bitcast(f16).rearrange(
                        "p (n two) -> p n two", two=2
                    )[:, :, 1]
                    nc.scalar.activation(
                        hi, ps[:], mybir.ActivationFunctionType.Copy,
                        bias=BIAS, scale=1.0,
                    )
                    nc.vector.max(pacc[m][:, c * 8:(c + 1) * 8], sj[:])

            for m in range(MTILES):
                nc.sync.dma_start(packed.ap()[m * 128:(m + 1) * 128, :], pacc[m][:])

    nc.compile()
    return nc


def _get_nc():
    if "nc" not in _NC_CACHE:
        _NC_CACHE["nc"] = _build_nc()
    return _NC_CACHE["nc"]


def _make_in_maps(queries, candidates):
    import ml_dtypes

    bf = ml_dtypes.bfloat16
    q = np.asarray(queries, dtype=np.float32)
    cand = np.asarray(candidates, dtype=np.float32)
    qTh = np.ascontiguousarray(q.T.astype(bf))  # [D, B] bf16
    in_maps = []
    for i in range(N_CORES):
        cTi = np.zeros((D, PADN), dtype=bf)
        cTi[:, :SHARD] = cand[i * SHARD:(i + 1) * SHARD].T.astype(bf)
        in_maps.append({"qT": qTh, "cT": cTi})
    return in_maps


def _run_device(in_maps, trace=False):
    from concourse import bass_utils

    nc = _get_nc()
    return bass_utils.run_bass_kernel_spmd(
        nc, in_maps, core_ids=list(range(N_CORES)), trace=trace
    )


def _merge(results, queries, candidates, identifiers, num_candidates):
    K = int(num_candidates)
    q = np.asarray(queries, dtype=np.float32)
    cand = np.asarray(candidates, dtype=np.float32)
    chunk_base = np.repeat(np.arange(NCHUNK, dtype=np.int64) * PCHUNK, 8)  # [SURV]
    all_u = []
    all_g = []
    for i in range(N_CORES):
        u = np.asarray(results[i]["packed"]).view(np.uint32)       # [B, SURV]
        local = chunk_base[None, :] + (u & 0xFFFF)                 # [B, SURV]
        valid = local < SHARD
        u = np.where(valid, u, 0)  # pads rank last
        g = i * SHARD + np.minimum(local, SHARD - 1)
        all_u.append(u)
        all_g.append(g)
    ucat = np.concatenate(all_u, axis=1)   # [B, 8*SURV] packed (monotone rank)
    gcat = np.concatenate(all_g, axis=1)
    # candidate set for exact rescoring: top RESCORE per row by packed rank
    nres = min(RESCORE, ucat.shape[1])
    part = np.argpartition(ucat, ucat.shape[1] - nres, axis=1)[:, -nres:]
    rows = np.arange(B)[:, None]
    gsel = gcat[rows, part]                                        # [B, nres]
    # exact fp32 rescore: s[b, j] = q[b] . cand[gsel[b, j]]
    csel = cand[gsel]                                              # [B, nres, D]
    vsel = np.einsum("bjd,bd->bj", csel, q, dtype=np.float32)
    # exact top-K, ties -> lower global index (matches lax.top_k)
    order = np.lexsort((gsel, -vsel), axis=-1)[:, :K]
    out_vals = np.take_along_axis(vsel, order, axis=1).astype(np.float32)
    out_gidx = np.take_along_axis(gsel, order, axis=1)
    ids = np.asarray(identifiers)
    out_ids = np.take(ids, out_gidx, axis=0)
    return out_vals, out_ids


def kernel(queries, candidates, identifiers, num_candidates):
    in_maps = _make_in_maps(queries, candidates)
    res = _run_device(in_maps, trace=False)
    return _merge(res.results, queries, candidates, identifiers, num_candidates)
